# revision 6
# baseline (speedup 1.0000x reference)
"""Grouped MLP (MoE expert-parallel) Trainium2 kernel, v2.

Expert-parallel: core e computes expert e's contiguous 16384-token block.

v2 vs v1: the x-transpose moved off the PE onto the DMA xbar
(dma_start_transpose, bf16 SBUF->SBUF), eliminating 1024 PE transposes +
256 ACT copies per core and the PSUM transpose staging. HW-verified: the
xbar fills xT[p, ho, t] with h-row h = ho*128+p — the same tiled layout
the baseline built on the PE — so the w1 SBUF layout is unchanged.

Structure (bf16 compute, fp32 PSUM accumulate, fp32 I/O), per 512-token
group: one 2MB cast-load (SWDGE) -> 4 xbar transposes (ACT HWDGE ring)
-> mm1 per gate/up pair (16 matmuls N=512 into 2 PSUM banks) -> SwiGLU
(ACT Silu + DVE mul) -> h [128i, 4is, 512t] bf16 -> mm2 (32 matmuls
N=512) -> PSUM evacuation split ACT/DVE -> one 2MB store (SP HWDGE
ring). Software-pipelined across 32 groups; weight DMAs issue after the
first x load.

PE work/group: 96 matmuls x 512cyc = 49152 cyc ~ 20.5us warm; ~655us/core.
"""

import numpy as np

E = 8
H = 1024
I = 512
T_PER_CORE = 16384
N_CORES = 8

# xbar transpose row-order (HW-verified): xT row (p,ho) holds h = ho*128+p
H_ORDER = "ho_p"

_cache = {}


def _build_nc(n_tiles):
    import concourse.mybir as mybir
    import concourse.tile as tile
    from concourse import bacc

    f32 = mybir.dt.float32
    bf16 = mybir.dt.bfloat16

    assert n_tiles % 4 == 0
    n_groups = n_tiles // 4

    nc = bacc.Bacc(None, target_bir_lowering=False)
    n_tok = n_tiles * 128
    x = nc.dram_tensor("x", [n_tok, H], f32, kind="ExternalInput")
    w1 = nc.dram_tensor("w1", [H, 2 * I], f32, kind="ExternalInput")
    w2 = nc.dram_tensor("w2", [I, H], f32, kind="ExternalInput")
    out = nc.dram_tensor("out", [n_tok, H], f32, kind="ExternalOutput")

    KT = H // 128  # 8 contraction tiles for mm1

    with tile.TileContext(nc) as tc:
        with (
            tc.tile_pool(name="const", bufs=1) as const,
            tc.tile_pool(name="xin", bufs=3) as xin,
            tc.tile_pool(name="xtp", bufs=3) as xtp,
            tc.tile_pool(name="actp", bufs=4) as actp,
            tc.tile_pool(name="hp", bufs=3) as hp,
            tc.tile_pool(name="outp", bufs=3) as outp,
            tc.tile_pool(name="mm1_ps", bufs=4, space="PSUM") as mm1_ps_pool,
            tc.tile_pool(name="mm2_ps", bufs=4, space="PSUM") as mm2_ps_pool,
        ):
            # Resident weights (bf16 via SWDGE cast-load). w1 rows follow the
            # xbar's xT row order so mm1 k-tiles contract matching h-subsets.
            w1_sb = const.tile([128, KT, 2 * I], bf16)
            w2_sb = const.tile([128, I // 128, H], bf16)

            def load_weights():
                w1_ap = w1.ap().rearrange("(ho p) f -> p ho f", p=128)
                # Split so the f-columns mm1's first gate/up pair reads
                # ([0:128] and [512:640]) land first and mm1 of group 0
                # starts ~4x earlier; Tile range-tracks w1_sb so each
                # matmul only waits on the slices it reads.
                nc.gpsimd.dma_start(w1_sb[:, :, 0:128], w1_ap[:, :, 0:128])
                nc.gpsimd.dma_start(w1_sb[:, :, 512:640], w1_ap[:, :, 512:640])
                nc.gpsimd.dma_start(w1_sb[:, :, 128:512], w1_ap[:, :, 128:512])
                nc.gpsimd.dma_start(w1_sb[:, :, 640:1024], w1_ap[:, :, 640:1024])
                nc.gpsimd.dma_start(
                    w2_sb[:], w2.ap().rearrange("(io p) f -> p io f", p=128)
                )

            xT_d, h_d = {}, {}

            def stage_load(g):
                x_t = xin.tile([128, 4, H], bf16, tag="x")
                nc.gpsimd.dma_start(
                    x_t[:],
                    x.ap()[g * 512 : (g + 1) * 512, :].rearrange(
                        "(tp p) h -> p tp h", p=128
                    ),
                )
                return x_t

            def stage_transpose(g, x_t):
                xT = xtp.tile([128, KT, 512], bf16, tag="xT")
                for tp in range(4):
                    nc.scalar.dma_start(
                        xT[:, :, tp * 128 : (tp + 1) * 128],
                        x_t[:, tp, :],
                        transpose=True,
                    )
                xT_d[g] = xT

            def stage_mm1_swiglu(g):
                xT = xT_d.pop(g)
                h = hp.tile([128, I // 128, 512], bf16, tag="h")
                for j in range(4):
                    ps_pair = []
                    for fo in (j, j + 4):
                        ps = mm1_ps_pool.tile([128, 512], f32, tag="mm1")
                        for ks in range(KT):
                            nc.tensor.matmul(
                                ps[:],
                                w1_sb[:, ks, fo * 128 : (fo + 1) * 128],
                                xT[:, ks, :],
                                start=(ks == 0),
                                stop=(ks == KT - 1),
                            )
                        ps_pair.append(ps)
                    gate_ps, up_ps = ps_pair
                    s = actp.tile([128, 512], f32, tag="s")
                    nc.scalar.activation(
                        s[:], gate_ps[:], mybir.ActivationFunctionType.Silu
                    )
                    nc.vector.tensor_mul(h[:, j, :], s[:], up_ps[:])
                h_d[g] = h

            def stage_mm2_store(g):
                h = h_d.pop(g)
                o_t = outp.tile([128, 4, H], f32, tag="o")
                for tp in range(4):
                    for hc in range(2):
                        ps2 = mm2_ps_pool.tile([128, 512], f32, tag="mm2")
                        for is_ in range(I // 128):
                            nc.tensor.matmul(
                                ps2[:],
                                h[:, is_, tp * 128 : (tp + 1) * 128],
                                w2_sb[:, is_, hc * 512 : (hc + 1) * 512],
                                start=(is_ == 0),
                                stop=(is_ == I // 128 - 1),
                            )
                        # All PSUM evacuation on DVE: the ACT sequencer also
                        # dispatches the xbar-transpose DMAs (strict-FIFO
                        # queue), so keeping ACT free of copies removes
                        # dispatch jitter from the xT critical path.
                        dst = o_t[:, tp, hc * 512 : (hc + 1) * 512]
                        nc.vector.tensor_copy(dst, ps2[:])
                nc.sync.dma_start(
                    out.ap()[g * 512 : (g + 1) * 512, :].rearrange(
                        "(tp p) h -> p tp h", p=128
                    ),
                    o_t[:],
                )

            for i in range(n_groups + 2):
                if i < n_groups:
                    x_t = stage_load(i)
                    if i == 0:
                        load_weights()
                    stage_transpose(i, x_t)
                if 1 <= i <= n_groups:
                    stage_mm1_swiglu(i - 1)
                if 2 <= i <= n_groups + 1:
                    stage_mm2_store(i - 2)

    nc.compile()
    return nc


def _get_nc(n_tiles):
    if n_tiles not in _cache:
        _cache[n_tiles] = _build_nc(n_tiles)
    return _cache[n_tiles]


def kernel(hidden_states, gate_up_proj, down_proj, num_tokens_per_expert):
    sizes = np.asarray(num_tokens_per_expert)
    offsets = np.concatenate([[0], np.cumsum(sizes)])
    uniform = (
        sizes.shape[0] == E
        and np.all(sizes == T_PER_CORE)
        and hidden_states.shape == (E * T_PER_CORE, H)
    )
    if not uniform:
        outs = []
        for e in range(sizes.shape[0]):
            xe = hidden_states[offsets[e] : offsets[e + 1]].astype(np.float32)
            merged = xe @ gate_up_proj[e]
            gate, up = merged[:, :I], merged[:, I:]
            he = (gate / (1.0 + np.exp(-gate))) * up
            outs.append(he @ down_proj[e])
        return np.concatenate(outs, axis=0).astype(hidden_states.dtype)

    from concourse.bass_utils import run_bass_kernel_spmd

    nc = _get_nc(T_PER_CORE // 128)
    hs = np.ascontiguousarray(np.asarray(hidden_states, dtype=np.float32))
    w1 = np.ascontiguousarray(np.asarray(gate_up_proj, dtype=np.float32))
    w2 = np.ascontiguousarray(np.asarray(down_proj, dtype=np.float32))
    in_maps = [
        {
            "x": hs[e * T_PER_CORE : (e + 1) * T_PER_CORE],
            "w1": w1[e],
            "w2": w2[e],
        }
        for e in range(N_CORES)
    ]
    res = run_bass_kernel_spmd(nc, in_maps, core_ids=list(range(N_CORES)))
    return np.concatenate([r["out"] for r in res.results], axis=0)


# revision 7
# speedup vs baseline: 2.4170x; 2.4170x over previous
"""Grouped MLP (MoE expert-parallel) Trainium2 kernel, v2.

Expert-parallel: core e computes expert e's contiguous 16384-token block.

v2 vs v1: the x-transpose moved off the PE onto the DMA xbar
(dma_start_transpose, bf16 SBUF->SBUF), eliminating 1024 PE transposes +
256 ACT copies per core and the PSUM transpose staging. HW-verified: the
xbar fills xT[p, ho, t] with h-row h = ho*128+p — the same tiled layout
the baseline built on the PE — so the w1 SBUF layout is unchanged.

Structure (bf16 compute, fp32 PSUM accumulate, fp32 I/O), per 512-token
group: one 2MB cast-load (SWDGE) -> 4 xbar transposes (ACT HWDGE ring)
-> mm1 per gate/up pair (16 matmuls N=512 into 2 PSUM banks) -> SwiGLU
(ACT Silu + DVE mul) -> h [128i, 4is, 512t] bf16 -> mm2 (32 matmuls
N=512) -> PSUM evacuation split ACT/DVE -> one 2MB store (SP HWDGE
ring). Software-pipelined across 32 groups; weight DMAs issue after the
first x load.

PE work/group: 96 matmuls x 512cyc = 49152 cyc ~ 20.5us warm; ~655us/core.
"""

import numpy as np

E = 8
H = 1024
I = 512
T_PER_CORE = 16384
N_CORES = 8

# xbar transpose row-order (HW-verified): xT row (p,ho) holds h = ho*128+p
H_ORDER = "ho_p"

_cache = {}


def _build_nc(n_tiles):
    import concourse.mybir as mybir
    import concourse.tile as tile
    from concourse import bacc

    f32 = mybir.dt.float32
    bf16 = mybir.dt.bfloat16

    assert n_tiles % 4 == 0
    n_groups = n_tiles // 4

    nc = bacc.Bacc(None, target_bir_lowering=False)
    n_tok = n_tiles * 128
    x = nc.dram_tensor("x", [n_tok, H], f32, kind="ExternalInput")
    w1 = nc.dram_tensor("w1", [H, 2 * I], f32, kind="ExternalInput")
    w2 = nc.dram_tensor("w2", [I, H], f32, kind="ExternalInput")
    out = nc.dram_tensor("out", [n_tok, H], f32, kind="ExternalOutput")

    KT = H // 128  # 8 contraction tiles for mm1

    with tile.TileContext(nc) as tc:
        with (
            tc.tile_pool(name="const", bufs=1) as const,
            tc.tile_pool(name="xin", bufs=3) as xin,
            tc.tile_pool(name="xtp", bufs=3) as xtp,
            tc.tile_pool(name="actp", bufs=4) as actp,
            tc.tile_pool(name="hp", bufs=2) as hp,
            tc.tile_pool(name="outp", bufs=3) as outp,
            tc.tile_pool(name="mm1_ps", bufs=4, space="PSUM") as mm1_ps_pool,
            tc.tile_pool(name="mm2_ps", bufs=4, space="PSUM") as mm2_ps_pool,
        ):
            # Resident weights (bf16 via SWDGE cast-load). w1 rows follow the
            # xbar's xT row order so mm1 k-tiles contract matching h-subsets.
            w1_sb = const.tile([128, KT, 2 * I], bf16)
            w2_sb = const.tile([128, I // 128, H], bf16)

            def load_weights():
                w1_ap = w1.ap().rearrange("(ho p) f -> p ho f", p=128)
                # Split so the f-columns mm1's first gate/up pair reads
                # ([0:128] and [512:640]) land first and mm1 of group 0
                # starts ~4x earlier; Tile range-tracks w1_sb so each
                # matmul only waits on the slices it reads.
                nc.gpsimd.dma_start(w1_sb[:, :, 0:128], w1_ap[:, :, 0:128])
                nc.gpsimd.dma_start(w1_sb[:, :, 512:640], w1_ap[:, :, 512:640])
                nc.gpsimd.dma_start(w1_sb[:, :, 128:512], w1_ap[:, :, 128:512])
                nc.gpsimd.dma_start(w1_sb[:, :, 640:1024], w1_ap[:, :, 640:1024])
                nc.gpsimd.dma_start(
                    w2_sb[:], w2.ap().rearrange("(io p) f -> p io f", p=128)
                )

            xT_d, h_d = {}, {}

            def stage_load(g):
                x_t = xin.tile([128, 4, H], bf16, tag="x")
                nc.gpsimd.dma_start(
                    x_t[:],
                    x.ap()[g * 512 : (g + 1) * 512, :].rearrange(
                        "(tp p) h -> p tp h", p=128
                    ),
                )
                return x_t

            def stage_transpose(g, x_t):
                xT = xtp.tile([128, KT, 512], bf16, tag="xT")
                for tp in range(4):
                    nc.scalar.dma_start(
                        xT[:, :, tp * 128 : (tp + 1) * 128],
                        x_t[:, tp, :],
                        transpose=True,
                    )
                xT_d[g] = xT

            def stage_mm1_swiglu(g):
                xT = xT_d.pop(g)
                h = hp.tile([128, I // 128, 512], bf16, tag="h")
                for j in range(4):
                    ps_pair = []
                    for fo in (j, j + 4):
                        ps = mm1_ps_pool.tile([128, 512], f32, tag="mm1")
                        for ks in range(KT):
                            nc.tensor.matmul(
                                ps[:],
                                w1_sb[:, ks, fo * 128 : (fo + 1) * 128],
                                xT[:, ks, :],
                                start=(ks == 0),
                                stop=(ks == KT - 1),
                            )
                        ps_pair.append(ps)
                    gate_ps, up_ps = ps_pair
                    s = actp.tile([128, 512], f32, tag="s")
                    nc.scalar.activation(
                        s[:], gate_ps[:], mybir.ActivationFunctionType.Silu
                    )
                    nc.vector.tensor_mul(h[:, j, :], s[:], up_ps[:])
                h_d[g] = h

            def stage_mm2_store(g):
                h = h_d.pop(g)
                o_t = outp.tile([128, 4, H], f32, tag="o")
                for tp in range(4):
                    for hc in range(2):
                        ps2 = mm2_ps_pool.tile([128, 512], f32, tag="mm2")
                        for is_ in range(I // 128):
                            nc.tensor.matmul(
                                ps2[:],
                                h[:, is_, tp * 128 : (tp + 1) * 128],
                                w2_sb[:, is_, hc * 512 : (hc + 1) * 512],
                                start=(is_ == 0),
                                stop=(is_ == I // 128 - 1),
                            )
                        dst = o_t[:, tp, hc * 512 : (hc + 1) * 512]
                        if hc == 0:
                            nc.scalar.copy(dst, ps2[:])
                        else:
                            nc.vector.tensor_copy(dst, ps2[:])
                nc.sync.dma_start(
                    out.ap()[g * 512 : (g + 1) * 512, :].rearrange(
                        "(tp p) h -> p tp h", p=128
                    ),
                    o_t[:],
                )

            for i in range(n_groups + 2):
                if i < n_groups:
                    x_t = stage_load(i)
                    if i == 0:
                        load_weights()
                    stage_transpose(i, x_t)
                if 1 <= i <= n_groups:
                    stage_mm1_swiglu(i - 1)
                if 2 <= i <= n_groups + 1:
                    stage_mm2_store(i - 2)

    nc.compile()
    return nc


def _get_nc(n_tiles):
    if n_tiles not in _cache:
        _cache[n_tiles] = _build_nc(n_tiles)
    return _cache[n_tiles]


def kernel(hidden_states, gate_up_proj, down_proj, num_tokens_per_expert):
    sizes = np.asarray(num_tokens_per_expert)
    offsets = np.concatenate([[0], np.cumsum(sizes)])
    uniform = (
        sizes.shape[0] == E
        and np.all(sizes == T_PER_CORE)
        and hidden_states.shape == (E * T_PER_CORE, H)
    )
    if not uniform:
        outs = []
        for e in range(sizes.shape[0]):
            xe = hidden_states[offsets[e] : offsets[e + 1]].astype(np.float32)
            merged = xe @ gate_up_proj[e]
            gate, up = merged[:, :I], merged[:, I:]
            he = (gate / (1.0 + np.exp(-gate))) * up
            outs.append(he @ down_proj[e])
        return np.concatenate(outs, axis=0).astype(hidden_states.dtype)

    from concourse.bass_utils import run_bass_kernel_spmd

    nc = _get_nc(T_PER_CORE // 128)
    hs = np.ascontiguousarray(np.asarray(hidden_states, dtype=np.float32))
    w1 = np.ascontiguousarray(np.asarray(gate_up_proj, dtype=np.float32))
    w2 = np.ascontiguousarray(np.asarray(down_proj, dtype=np.float32))
    in_maps = [
        {
            "x": hs[e * T_PER_CORE : (e + 1) * T_PER_CORE],
            "w1": w1[e],
            "w2": w2[e],
        }
        for e in range(N_CORES)
    ]
    res = run_bass_kernel_spmd(nc, in_maps, core_ids=list(range(N_CORES)))
    return np.concatenate([r["out"] for r in res.results], axis=0)
